# revision 1
# baseline (speedup 1.0000x reference)
"""Trainium2 Bass kernel for nn_NetDensity (RISA net density maps).

Math (per net n with pins P_n):
  bbox: xmin/xmax/ymin/ymax over pins
  wt = RISA[min(|P_n|,46)] * net_weights[n]
  ox[i] = clip(min(xmax, b_i+2) - max(xmin, b_i), 0)   b_i = 2*i, i<256
  oy[j] likewise
  ch = wt/dy (dy>0 else 0), cv = wt/dx
  H = sum_n (ch*ox) outer oy ;  V = sum_n (cv*ox) outer oy
  out = (|H|+|V|, H, V)

Sharding: nets (and their CSR pin segments) are sharded across the 8 cores;
each core computes private 256x256 H^T/V^T partial maps which are summed on
the host (the unshard step).

Device formulation per 128-net tile (nets on the K/partition axis):
  T1 = max(b - xmax, -2)            (= -min(xmax-b, 2))
  t2 = relu(xmin - b)
  Sx = T1 + t2                      (= -(ox before outer relu))
  A_H = relu(nch * Sx)   nch = -wt/dy   (relu commutes: ch*ox = relu(nch*Sx))
  A_V = relu(ncv * Sx)
  B   = relu(-Sy)                   (= oy)
  PSUM += B_chunk^T @ [A_H | A_V]   -> [H^T | V^T]
"""

import numpy as np

import concourse.bass as bass
import concourse.bacc as bacc
import concourse.mybir as mybir
from concourse import tile
from concourse.bass_utils import run_bass_kernel_spmd

# Problem constants (fixed by the problem spec).
NUM_NETS = 262144
NUM_PINS = 1048576
NBX = 256
BSX = 2.0
NCORES = 8
NPC = NUM_NETS // NCORES          # nets per core: 32768
P = 128                            # SBUF partitions
NPP = NPC // P                     # nets per partition: 256
NTILES = NPP                       # one K-tile per net column: 256

_RISA_TAB = np.array(
    [1.0, 1.0, 1.0, 1.0,
     1.0828, 1.1536, 1.2206, 1.2823, 1.3385, 1.3991, 1.4493]
    + [1.6899] * 5 + [1.8924] * 5 + [2.0743] * 5 + [2.2334] * 5
    + [2.3892] * 5 + [2.5356] * 5 + [2.6625] * 5 + [2.7933],
    dtype=np.float32)

_CACHE = {}
TRACE = False          # test.py sets True to collect an NTFF profile
LAST_RESULT = None     # BassKernelResults of the most recent run


def _build(ntiles=NTILES):
    """Build + bacc-compile the per-core Bass program."""
    nets = P * ntiles
    f32 = mybir.dt.float32
    bf16 = mybir.dt.bfloat16

    nc = bacc.Bacc("TRN2", target_bir_lowering=False, debug=False,
                   num_devices=NCORES)
    # DRAM I/O. coords: partition p holds its nets' pin segments,
    # [p, net, pin(4), xy(2)] flattened to [128, ntiles*8].
    coords_d = nc.dram_tensor("coords", [P, ntiles * 8], f32, kind="ExternalInput")
    netw_d = nc.dram_tensor("netw", [P, ntiles], f32, kind="ExternalInput")
    nrisa_d = nc.dram_tensor("nrisa", [P, ntiles], f32, kind="ExternalInput")
    brow_d = nc.dram_tensor("brow", [P, NBX], f32, kind="ExternalInput")
    out_d = nc.dram_tensor("out", [2, P, 512], f32, kind="ExternalOutput")

    with tile.TileContext(nc) as tc:
        with (
            tc.tile_pool(name="const", bufs=1) as cpool,
            tc.tile_pool(name="scal", bufs=1) as spool,
            tc.tile_pool(name="work", bufs=6) as wpool,
            tc.tile_pool(name="psum", bufs=1, space="PSUM") as ppool,
        ):
            coords = cpool.tile([P, ntiles * 8], f32)
            netw = cpool.tile([P, ntiles], f32)
            nrisa = cpool.tile([P, ntiles], f32)
            brow = cpool.tile([P, NBX], f32)
            browb = cpool.tile([P, NBX], bf16)
            nc.sync.dma_start(out=coords[:], in_=coords_d[:, :])
            nc.sync.dma_start(out=netw[:], in_=netw_d[:, :])
            nc.sync.dma_start(out=nrisa[:], in_=nrisa_d[:, :])
            nc.sync.dma_start(out=brow[:], in_=brow_d[:, :])
            nc.vector.tensor_copy(out=browb[:], in_=brow[:])

            # ---- per-net scalars -------------------------------------
            # view coords as [P, net, pin, xy]
            c4 = coords[:].rearrange("p (n k t) -> p n k t", k=4, t=2)
            bbmax = spool.tile([P, ntiles * 2], f32)   # [p, net, (x,y)]
            bbmin = spool.tile([P, ntiles * 2], f32)
            ma = spool.tile([P, ntiles * 2], f32)
            mb = spool.tile([P, ntiles * 2], f32)
            mav = ma[:].rearrange("p (n t) -> p n t", t=2)
            mbv = mb[:].rearrange("p (n t) -> p n t", t=2)
            nc.vector.tensor_tensor(out=mav, in0=c4[:, :, 0, :], in1=c4[:, :, 1, :],
                                    op=mybir.AluOpType.max)
            nc.vector.tensor_tensor(out=mbv, in0=c4[:, :, 2, :], in1=c4[:, :, 3, :],
                                    op=mybir.AluOpType.max)
            nc.vector.tensor_tensor(out=bbmax[:], in0=ma[:], in1=mb[:],
                                    op=mybir.AluOpType.max)
            nc.vector.tensor_tensor(out=mav, in0=c4[:, :, 0, :], in1=c4[:, :, 1, :],
                                    op=mybir.AluOpType.min)
            nc.vector.tensor_tensor(out=mbv, in0=c4[:, :, 2, :], in1=c4[:, :, 3, :],
                                    op=mybir.AluOpType.min)
            nc.vector.tensor_tensor(out=bbmin[:], in0=ma[:], in1=mb[:],
                                    op=mybir.AluOpType.min)

            d = spool.tile([P, ntiles * 2], f32)       # (dx, dy) pairs
            nc.vector.tensor_tensor(out=d[:], in0=bbmax[:], in1=bbmin[:],
                                    op=mybir.AluOpType.subtract)
            dc = spool.tile([P, ntiles * 2], f32)
            nc.vector.tensor_scalar(out=dc[:], in0=d[:], scalar1=1e-12,
                                    scalar2=None, op0=mybir.AluOpType.max)
            rec = spool.tile([P, ntiles * 2], f32)
            nc.vector.reciprocal(out=rec[:], in_=dc[:])
            mask = spool.tile([P, ntiles * 2], f32)
            nc.vector.tensor_scalar(out=mask[:], in0=d[:], scalar1=0.0,
                                    scalar2=None, op0=mybir.AluOpType.is_gt)
            rm = spool.tile([P, ntiles * 2], f32)
            nc.vector.tensor_tensor(out=rm[:], in0=rec[:], in1=mask[:],
                                    op=mybir.AluOpType.mult)
            # negated combined weight -(risa * netw), broadcast to xy pairs
            nwt = spool.tile([P, ntiles], f32)
            nc.vector.tensor_tensor(out=nwt[:], in0=netw[:], in1=nrisa[:],
                                    op=mybir.AluOpType.mult)
            nwt2 = spool.tile([P, ntiles * 2], f32)
            nwt2v = nwt2[:].rearrange("p (n t) -> p n t", t=2)
            nc.vector.tensor_copy(out=nwt2v[:, :, 0], in_=nwt[:])
            nc.vector.tensor_copy(out=nwt2v[:, :, 1], in_=nwt[:])
            # nchv pairs: [.., 0] = -wt/dx = ncv ; [.., 1] = -wt/dy = nch
            nchv = spool.tile([P, ntiles * 2], f32)
            nc.vector.tensor_tensor(out=nchv[:], in0=rm[:], in1=nwt2[:],
                                    op=mybir.AluOpType.mult)

            ps0 = ppool.tile([P, 512], f32)
            ps1 = ppool.tile([P, 512], f32)

            # ---- main loop over net tiles ----------------------------
            for j in range(ntiles):
                xmax_j = bbmax[:, 2 * j:2 * j + 1]
                ymax_j = bbmax[:, 2 * j + 1:2 * j + 2]
                xmin_j = bbmin[:, 2 * j:2 * j + 1]
                ymin_j = bbmin[:, 2 * j + 1:2 * j + 2]
                ncv_j = nchv[:, 2 * j:2 * j + 1]
                nch_j = nchv[:, 2 * j + 1:2 * j + 2]

                TU = wpool.tile([P, 512], bf16, tag="TU")
                tu2 = wpool.tile([P, 512], bf16, tag="tu2")
                Sxy = wpool.tile([P, 512], bf16, tag="Sxy")
                AHV = wpool.tile([P, 512], bf16, tag="AHV")
                Bt = wpool.tile([P, NBX], bf16, tag="Bt")

                # T1 = max(b - xmax, -2) ; U1 = max(b - ymax, -2)   [DVE]
                nc.vector.tensor_scalar(out=TU[:, 0:256], in0=browb[:],
                                        scalar1=xmax_j, scalar2=-2.0,
                                        op0=mybir.AluOpType.subtract,
                                        op1=mybir.AluOpType.max)
                nc.vector.tensor_scalar(out=TU[:, 256:512], in0=browb[:],
                                        scalar1=ymax_j, scalar2=-2.0,
                                        op0=mybir.AluOpType.subtract,
                                        op1=mybir.AluOpType.max)
                # t2 = relu(xmin - b) ; u2 = relu(ymin - b)   [ACT]
                nc.scalar.activation(out=tu2[:, 0:256], in_=browb[:],
                                     func=mybir.ActivationFunctionType.Relu,
                                     bias=xmin_j, scale=-1.0)
                nc.scalar.activation(out=tu2[:, 256:512], in_=browb[:],
                                     func=mybir.ActivationFunctionType.Relu,
                                     bias=ymin_j, scale=-1.0)
                # Sx|Sy = TU + tu2 (one 512-wide op)   [DVE]
                nc.vector.tensor_tensor(out=Sxy[:], in0=TU[:], in1=tu2[:],
                                        op=mybir.AluOpType.add)
                # A_H = relu(nch * Sx)   [DVE]
                nc.vector.tensor_scalar(out=AHV[:, 0:256], in0=Sxy[:, 0:256],
                                        scalar1=nch_j, scalar2=0.0,
                                        op0=mybir.AluOpType.mult,
                                        op1=mybir.AluOpType.max)
                # A_V = relu(ncv * Sx)   [DVE]
                nc.vector.tensor_scalar(out=AHV[:, 256:512], in0=Sxy[:, 0:256],
                                        scalar1=ncv_j, scalar2=0.0,
                                        op0=mybir.AluOpType.mult,
                                        op1=mybir.AluOpType.max)
                # B = oy = relu(-Sy)   [ACT]
                nc.scalar.activation(out=Bt[:], in_=Sxy[:, 256:512],
                                     func=mybir.ActivationFunctionType.Relu,
                                     scale=-1.0)

                nc.tensor.matmul(out=ps0[:], lhsT=Bt[:, 0:128], rhs=AHV[:],
                                 start=(j == 0), stop=(j == ntiles - 1))
                nc.tensor.matmul(out=ps1[:], lhsT=Bt[:, 128:256], rhs=AHV[:],
                                 start=(j == 0), stop=(j == ntiles - 1))

            # ---- write out -------------------------------------------
            o0 = cpool.tile([P, 512], f32, tag="o0")
            o1 = cpool.tile([P, 512], f32, tag="o1")
            nc.vector.tensor_copy(out=o0[:], in_=ps0[:])
            nc.vector.tensor_copy(out=o1[:], in_=ps1[:])
            nc.sync.dma_start(out=out_d[0, :, :], in_=o0[:])
            nc.sync.dma_start(out=out_d[1, :, :], in_=o1[:])

    nc.compile()
    return nc


def _shard_inputs(pin_pos, netpin_start, flat_netpin, net_weights, ntiles=NTILES):
    """Host-side sharding: nets (and their CSR pin segments) across 8 cores."""
    nets = P * ntiles
    xy = np.asarray(pin_pos, dtype=np.float32).reshape(-1, 2)
    nps = np.asarray(netpin_start, dtype=np.int64)
    fnp = np.asarray(flat_netpin, dtype=np.int64)
    nw = np.asarray(net_weights, dtype=np.float32)

    cnt_all = nps[1:] - nps[:-1]
    nrisa_all = -_RISA_TAB[np.minimum(cnt_all, len(_RISA_TAB) - 1)]

    brow = np.broadcast_to(
        (np.arange(NBX, dtype=np.float32) * BSX)[None, :], (P, NBX)).copy()

    in_maps = []
    for c in range(NCORES):
        lo = c * nets
        sel = np.arange(lo, lo + nets)
        # pad each net's pin list to 4 by repeating its first pin
        # (doesn't change the bbox)
        starts = nps[sel]
        cnts = np.maximum(cnt_all[sel], 1)
        k = np.minimum(np.arange(4)[None, :], (cnts - 1)[:, None])
        pin_ids = fnp[starts[:, None] + k]              # [nets, 4]
        coords = xy[pin_ids.reshape(-1)]                # [nets*4, 2]
        in_maps.append({
            "coords": np.ascontiguousarray(coords.reshape(P, ntiles * 8)),
            "netw": np.ascontiguousarray(nw[sel].reshape(P, ntiles)),
            "nrisa": np.ascontiguousarray(nrisa_all[sel].reshape(P, ntiles)),
            "brow": brow,
        })
    return in_maps


def kernel(pin_pos, netpin_start, flat_netpin, net_weights):
    key = NTILES
    if key not in _CACHE:
        _CACHE[key] = _build(NTILES)
    nc = _CACHE[key]

    in_maps = _shard_inputs(pin_pos, netpin_start, flat_netpin, net_weights)
    res = run_bass_kernel_spmd(nc, in_maps, core_ids=list(range(NCORES)),
                               trace=TRACE)
    global LAST_RESULT
    LAST_RESULT = res

    # Unshard: sum the per-core partial transposed maps, then transpose.
    HT = np.zeros((256, 256), dtype=np.float32)
    VT = np.zeros((256, 256), dtype=np.float32)
    for c in range(NCORES):
        o = res.results[c]["out"]          # [2, 128, 512]
        HT[0:128] += o[0, :, 0:256]
        HT[128:256] += o[1, :, 0:256]
        VT[0:128] += o[0, :, 256:512]
        VT[128:256] += o[1, :, 256:512]
    H = np.ascontiguousarray(HT.T)
    V = np.ascontiguousarray(VT.T)
    return np.abs(H) + np.abs(V), H, V

